# revision 1
# baseline (speedup 1.0000x reference)
"""Trainium2 Bass kernel for a dense transformer block (nn_Block_67147518706214).

Sharding: data-parallel over batch. 64 images are split 8-per-core across the
8 NeuronCores; weights are replicated. No collectives.

Device layout is channel-major [channels(partitions), tokens(free)]. The host
pre-transposes x per core to (768, 8*196) and transposes the kernel's
channel-major output back. LayerNorm gains/biases are folded into the weight
matrices on the host, so the device only ever computes z = (x - mu) * rstd.
"""

import sys
from contextlib import ExitStack

sys.path.insert(0, "/opt/trn_rl_repo")

import numpy as np

import concourse.bass as bass
import concourse.tile as tile
from concourse import bacc, mybir
from concourse.bass_utils import run_bass_kernel_spmd

f32 = mybir.dt.float32
f32r = mybir.dt.float32r
AF = mybir.ActivationFunctionType
OP = mybir.AluOpType
AX = mybir.AxisListType

# Problem shapes (hardcoded per spec)
DIM = 768
HID = 64
MLP = 3072
N_TOK = 196
B_TOTAL = 64
N_CORES = 8
B = B_TOTAL // N_CORES            # images per core
HW = 14                           # 14x14 spatial grid
EPS = 1e-5

P = 128
CT = DIM // P                     # 6 channel tiles
IMG_PER_CHUNK = 2
T2 = IMG_PER_CHUNK * N_TOK        # 392 tokens per chunk
NCHUNK = B // IMG_PER_CHUNK       # 4 chunks per core
TOK_ALL = B * N_TOK               # 1568 tokens per core
MT_KV = 2 * DIM // P              # 12 output tiles for kv
JW = 1536                         # MLP streaming chunk width
NJ = MLP // JW                    # 2 chunks
PT = JW // P                      # 12 partition tiles per MLP chunk

_CACHE = {}


def _ln_stats(nc, rows, ps_stats, ps_bcast, onk, onm, eps11, x_tiles, xsq_tiles):
    """Channel-major LN stats. x_tiles/xsq_tiles: lists of CT [P, T2] f32r APs.
    Returns (MU, R): PSUM [P, T2] per-token mean/rstd broadcast to all parts."""
    psum_s = ps_stats.tile([1, T2], f32, tag="psum_s")
    psum_q = ps_stats.tile([1, T2], f32, tag="psum_q")
    for kt in range(CT):
        nc.tensor.matmul(psum_s, onk, x_tiles[kt], start=(kt == 0), stop=(kt == CT - 1))
    for kt in range(CT):
        nc.tensor.matmul(psum_q, onk, xsq_tiles[kt], start=(kt == 0), stop=(kt == CT - 1))
    mu_row = rows.tile([1, T2], f32r, tag="mu_row")
    nc.scalar.activation(out=mu_row, in_=psum_s, func=AF.Copy, bias=0.0, scale=1.0 / DIM)
    # broadcast MU as soon as mu_row exists: the z-subtract needs only MU
    MU = ps_bcast.tile([P, T2], f32, tag="MU")
    nc.tensor.matmul(MU, onm, mu_row, start=True, stop=True)
    musq_row = rows.tile([1, T2], f32, tag="musq_row")
    nc.vector.tensor_mul(musq_row, mu_row.bitcast(f32), mu_row.bitcast(f32))
    var_row = rows.tile([1, T2], f32, tag="var_row")
    nc.vector.scalar_tensor_tensor(out=var_row, in0=psum_q, scalar=1.0 / DIM,
                                   in1=musq_row, op0=OP.mult, op1=OP.subtract)
    sd_row = rows.tile([1, T2], f32, tag="sd_row")
    nc.scalar.activation(out=sd_row, in_=var_row, func=AF.Sqrt, bias=eps11, scale=1.0)
    r_row = rows.tile([1, T2], f32r, tag="r_row")
    with nc.allow_low_precision(reason="fp32r rstd row feeds fp32r broadcast matmul"):
        nc.vector.reciprocal(out=r_row, in_=sd_row)
    R = ps_bcast.tile([P, T2], f32, tag="R")
    nc.tensor.matmul(R, onm, r_row, start=True, stop=True)
    return MU, R


def build_module():
    nc = bacc.Bacc("TRN2", target_bir_lowering=False, debug=False, enable_asserts=True)

    # ---- DRAM I/O ----
    xT_d = nc.dram_tensor("xT", [DIM, TOK_ALL], f32, kind="ExternalInput").ap()
    kvW_d = nc.dram_tensor("kvW", [DIM, 2 * DIM], f32, kind="ExternalInput").ap()
    qW1_d = nc.dram_tensor("qW1", [DIM, HID], f32, kind="ExternalInput").ap()
    qW2_d = nc.dram_tensor("qW2", [HID, DIM], f32, kind="ExternalInput").ap()
    projW_d = nc.dram_tensor("projW", [DIM, DIM], f32, kind="ExternalInput").ap()
    fc1W_d = nc.dram_tensor("fc1W", [DIM, MLP], f32, kind="ExternalInput").ap()
    fc2W_d = nc.dram_tensor("fc2W", [MLP, DIM], f32, kind="ExternalInput").ap()
    posW_d = nc.dram_tensor("posW", [DIM, 9], f32, kind="ExternalInput").ap()
    sepeb_d = nc.dram_tensor("sepeb", [DIM, N_TOK], f32, kind="ExternalInput").ap()
    bkv_d = nc.dram_tensor("bkv", [MT_KV, P], f32, kind="ExternalInput").ap()
    bu_d = nc.dram_tensor("bu", [HID, 1], f32, kind="ExternalInput").ap()
    bprj_d = nc.dram_tensor("bprj", [CT, P], f32, kind="ExternalInput").ap()
    bfc1_d = nc.dram_tensor("bfc1", [MLP // P, P], f32, kind="ExternalInput").ap()
    ones_d = nc.dram_tensor("ones", [P, 2], f32, kind="ExternalInput").ap()

    yT_d = nc.dram_tensor("yT", [DIM, TOK_ALL], f32, kind="ExternalOutput").ap()

    with tile.TileContext(nc) as tc:
        _body(nc, tc, xT_d, kvW_d, qW1_d, qW2_d, projW_d, fc1W_d, fc2W_d,
              posW_d, sepeb_d, bkv_d, bu_d, bprj_d, bfc1_d, ones_d, yT_d)
    nc.compile()
    return nc


def _body(nc, tc, xT_d, kvW_d, qW1_d, qW2_d, projW_d, fc1W_d, fc2W_d,
          posW_d, sepeb_d, bkv_d, bu_d, bprj_d, bfc1_d, ones_d, yT_d):
    with ExitStack() as root:
        statics = root.enter_context(tc.tile_pool(name="statics", bufs=1))
        rows = root.enter_context(tc.tile_pool(name="rows", bufs=1))
        small = root.enter_context(tc.tile_pool(name="small", bufs=3))

        # ---- static loads ----
        ones2 = statics.tile([P, 2], f32r)
        nc.sync.dma_start(out=ones2, in_=ones_d.bitcast(f32r))
        onk = ones2[:, 0:1]           # [128,1] ones column (K dim)
        onm = statics.tile([1, P], f32r)
        nc.sync.dma_start(out=onm, in_=ones_d[:, 0:1].rearrange("p a -> a p").bitcast(f32r))

        bkv_sb = statics.tile([P, MT_KV], f32)
        nc.sync.dma_start(out=bkv_sb, in_=bkv_d.rearrange("m p -> p m"))
        bu_sb = statics.tile([HID, 1], f32)
        nc.sync.dma_start(out=bu_sb, in_=bu_d)
        bprj_sb = statics.tile([P, CT], f32)
        nc.sync.dma_start(out=bprj_sb, in_=bprj_d.rearrange("m p -> p m"))
        bfc1_sb = statics.tile([P, MLP // P], f32)
        nc.sync.dma_start(out=bfc1_sb, in_=bfc1_d.rearrange("m p -> p m"))

        eps11 = statics.tile([1, 1], f32)
        nc.vector.memset(eps11, EPS)

        # residual stream x1^T
        x1 = statics.tile([P, CT, TOK_ALL], f32r)

        # ---- Phase A: LN1 + attention, per 2-image chunk ----
        with ExitStack() as phase_a:
            wA = phase_a.enter_context(tc.tile_pool(name="wA", bufs=1))
            chk = phase_a.enter_context(tc.tile_pool(name="chk", bufs=1))
            chkx = phase_a.enter_context(tc.tile_pool(name="chkx", bufs=2))
            convst = phase_a.enter_context(tc.tile_pool(name="convst", bufs=1))
            ps_stats = phase_a.enter_context(tc.tile_pool(name="ps_stats", bufs=1, space="PSUM"))
            ps_bcast = phase_a.enter_context(tc.tile_pool(name="ps_bcast", bufs=1, space="PSUM"))
            ps_mm = phase_a.enter_context(tc.tile_pool(name="ps_mm", bufs=4, space="PSUM"))

            posW_sb = convst.tile([P, CT, 9], f32)
            nc.sync.dma_start(out=posW_sb, in_=posW_d.rearrange("(ct p) k -> p ct k", p=P))
            sepeb_sb = convst.tile([P, CT, N_TOK], f32)
            nc.sync.dma_start(out=sepeb_sb, in_=sepeb_d.rearrange("(ct p) t -> p ct t", p=P))
            # conv padded buffer (borders stay zero forever)
            zpad = convst.tile([P, CT, IMG_PER_CHUNK, 16, 16], f32)
            nc.vector.memset(zpad, 0.0)

            def dma_xc(t, ch):
                # per-ct pieces: downstream per-ct consumers start sooner
                for ct in range(CT):
                    nc.sync.dma_start(
                        out=t[:, ct],
                        in_=xT_d.rearrange("(ct p) t -> p ct t", p=P)
                            [:, ct, ch * T2:(ch + 1) * T2].bitcast(f32r))

            # chunk-0 x first, then kv weights (first consumer), then the rest
            xc_pre = []
            t0 = chkx.tile([P, CT, T2], f32r, tag="xc")
            dma_xc(t0, 0)
            xc_pre.append(t0)

            kvW = wA.tile([P, CT, 2 * DIM], f32r)
            for kt in range(CT):
                nc.sync.dma_start(
                    out=kvW[:, kt],
                    in_=kvW_d.rearrange("(kt p) m -> p kt m", p=P)[:, kt].bitcast(f32r))
            qW1 = wA.tile([P, CT, HID], f32r)
            nc.sync.dma_start(out=qW1, in_=qW1_d.rearrange("(kt p) m -> p kt m", p=P).bitcast(f32r))
            qW2 = wA.tile([HID, DIM], f32r)
            nc.sync.dma_start(out=qW2, in_=qW2_d.bitcast(f32r))
            t1 = chkx.tile([P, CT, T2], f32r, tag="xc")
            dma_xc(t1, 1)
            xc_pre.append(t1)
            projW = wA.tile([P, CT, DIM], f32r)
            for kt in range(CT):
                nc.sync.dma_start(
                    out=projW[:, kt],
                    in_=projW_d.rearrange("(kt p) m -> p kt m", p=P)[:, kt].bitcast(f32r))

            for ch in range(NCHUNK):
                tok0 = ch * T2
                if ch < 2:
                    xc = xc_pre[ch]
                else:
                    xc = chkx.tile([P, CT, T2], f32r, tag="xc")
                    dma_xc(xc, ch)

                # LN1 stats (xsq shares the "xz" slot with zt below)
                xsq = chk.tile([P, CT, T2], f32r, tag="xz")
                for ct in range(CT):
                    if ch == 0:
                        # split the first chunk's squares across ACT and DVE to
                        # shorten the startup serial chain
                        if ct % 2 == 1:
                            nc.vector.tensor_mul(xsq[:, ct], xc[:, ct].bitcast(f32),
                                                 xc[:, ct].bitcast(f32))
                        else:
                            nc.scalar.activation(out=xsq[:, ct], in_=xc[:, ct].bitcast(f32),
                                                 func=AF.Square, bias=0.0, scale=1.0)
                    else:
                        nc.scalar.activation(out=xsq[:, ct], in_=xc[:, ct].bitcast(f32),
                                             func=AF.Square, bias=0.0, scale=1.0)
                MU, R = _ln_stats(nc, rows, ps_stats, ps_bcast, onk, onm, eps11,
                                  [xc[:, ct] for ct in range(CT)],
                                  [xsq[:, ct] for ct in range(CT)])
                # z = (x - MU) * R   (sub on DVE from PSUM; mul on GpSimd from SBUF)
                R_sb = small.tile([P, T2], f32, tag="R_sb")
                nc.scalar.activation(out=R_sb, in_=R, func=AF.Copy, bias=0.0, scale=1.0)
                zt = chk.tile([P, CT, T2], f32r, tag="xz")
                for ct in range(CT):
                    nc.vector.tensor_sub(zt[:, ct], xc[:, ct].bitcast(f32), MU)
                    nc.gpsimd.tensor_mul(zt[:, ct], zt[:, ct].bitcast(f32), R_sb)

                # depthwise 3x3 conv on z. Taps 0-6 accumulate on DVE into sepeT
                # (tap 0 folds in the bias image); taps 7-8 on GpSimd into sepeB
                # (no STT on Pool: mul, mul, add). Emitted in two batches around
                # the softmax DVE ops so the q*k0 path is not queued behind the
                # whole conv on DVE.
                sepeT = chk.tile([P, CT, IMG_PER_CHUNK, HW, HW], f32, tag="sepeT")
                sepeB = chk.tile([P, CT, IMG_PER_CHUNK, HW, HW], f32, tag="sepeB")
                for ct in range(CT):
                    for img in range(IMG_PER_CHUNK):
                        nc.gpsimd.tensor_copy(
                            out=zpad[:, ct, img, 1:15, 1:15],
                            in_=zt[:, ct, img * N_TOK:(img + 1) * N_TOK].bitcast(f32)
                                .rearrange("p (y x) -> p y x", y=HW))

                def emit_conv_ct(ct):
                    for img in range(IMG_PER_CHUNK):
                        dst = sepeT[:, ct, img]
                        dstB = sepeB[:, ct, img]
                        for tap in range(7):
                            ky, kx = tap // 3, tap % 3
                            win = zpad[:, ct, img, ky:ky + HW, kx:kx + HW]
                            wsc = posW_sb[:, ct, tap:tap + 1]
                            if tap == 0:
                                nc.vector.scalar_tensor_tensor(
                                    out=dst, in0=win, scalar=wsc,
                                    in1=sepeb_sb[:, ct, :]
                                        .rearrange("p (y x) -> p y x", y=HW),
                                    op0=OP.mult, op1=OP.add)
                            else:
                                nc.vector.scalar_tensor_tensor(
                                    out=dst, in0=win, scalar=wsc, in1=dst,
                                    op0=OP.mult, op1=OP.add)
                        tmpB = small.tile([P, HW, HW], f32, tag="tmpB")
                        nc.gpsimd.tensor_scalar_mul(
                            dstB, zpad[:, ct, img, 2:16, 1:15], posW_sb[:, ct, 7:8])
                        nc.gpsimd.tensor_scalar_mul(
                            tmpB, zpad[:, ct, img, 2:16, 2:16], posW_sb[:, ct, 8:9])
                        nc.gpsimd.tensor_add(dstB, dstB, tmpB)

                for ct in range(CT):
                    emit_conv_ct(ct)

                # kv = z @ kvW + bkv   (k0/k1 in separate slots so next chunk's
                # k0 work can start while this chunk still reads k1)
                k0T = chk.tile([P, CT, T2], f32, tag="k0T")
                k1T = chk.tile([P, CT, T2], f32, tag="k1T")
                for mt in range(MT_KV):
                    pk = ps_mm.tile([P, T2], f32, tag="mm")
                    for kt in range(CT):
                        nc.tensor.matmul(pk, kvW[:, kt, mt * P:(mt + 1) * P], zt[:, kt],
                                         start=(kt == 0), stop=(kt == CT - 1))
                    dst_kv = k0T[:, mt] if mt < CT else k1T[:, mt - CT]
                    nc.scalar.activation(out=dst_kv, in_=pk, func=AF.Identity,
                                         bias=bkv_sb[:, mt:mt + 1], scale=1.0)

                # u = gelu(z @ qW1 + bu)
                pu = ps_mm.tile([P, T2], f32, tag="mm")
                for kt in range(CT):
                    nc.tensor.matmul(pu[0:HID, :], qW1[:, kt], zt[:, kt],
                                     start=(kt == 0), stop=(kt == CT - 1))
                uT = chk.tile([HID, T2], f32r, tag="uT")
                nc.scalar.activation(out=uT, in_=pu[0:HID, :], func=AF.Gelu,
                                     bias=bu_sb, scale=1.0)

                # q = qW2.T @ u -> s = q * k0
                aT = chk.tile([P, CT, T2], f32r, tag="sT")
                sT = aT.bitcast(f32)
                for mt in range(CT):
                    pq = ps_mm.tile([P, T2], f32, tag="mm")
                    nc.tensor.matmul(pq, qW2[:, mt * P:(mt + 1) * P], uT,
                                     start=True, stop=True)
                    nc.vector.tensor_mul(aT[:, mt], pq, k0T[:, mt])

                # exp over tokens (no max-subtraction: |s| <= ~30, cannot
                # overflow fp32)
                zsum = small.tile([P, CT, IMG_PER_CHUNK], f32, tag="zsum")
                for ct in range(CT):
                    for img in range(IMG_PER_CHUNK):
                        seg = slice(img * N_TOK, (img + 1) * N_TOK)
                        nc.scalar.activation(out=aT[:, ct, seg], in_=sT[:, ct, seg],
                                             func=AF.Exp, bias=0.0, scale=1.0,
                                             accum_out=zsum[:, ct, img:img + 1])
                invZ = small.tile([P, CT, IMG_PER_CHUNK], f32, tag="invZ")
                for ct in range(CT):
                    nc.vector.reciprocal(out=invZ[:, ct, :], in_=zsum[:, ct, :])

                # a = (e * k1) * invZ + sepe  (in-place over the exp result)
                for ct in range(CT):
                    nc.gpsimd.tensor_mul(aT[:, ct], sT[:, ct], k1T[:, ct])
                    for img in range(IMG_PER_CHUNK):
                        seg = slice(img * N_TOK, (img + 1) * N_TOK)
                        nc.vector.scalar_tensor_tensor(
                            out=aT[:, ct, seg], in0=sT[:, ct, seg],
                            scalar=invZ[:, ct, img:img + 1],
                            in1=sepeT[:, ct, img].rearrange("p y x -> p (y x)"),
                            op0=OP.mult, op1=OP.add)
                        nc.vector.tensor_add(
                            aT[:, ct, seg], sT[:, ct, seg],
                            sepeB[:, ct, img].rearrange("p y x -> p (y x)"))

                # x1 = x + a @ projW + (projB + fc2b)
                for mt in range(CT):
                    pp = ps_mm.tile([P, T2], f32, tag="mm")
                    for kt in range(CT):
                        nc.tensor.matmul(pp, projW[:, kt, mt * P:(mt + 1) * P], aT[:, kt],
                                         start=(kt == 0), stop=(kt == CT - 1))
                    nc.vector.scalar_tensor_tensor(
                        out=x1[:, mt, tok0:tok0 + T2], in0=pp,
                        scalar=bprj_sb[:, mt:mt + 1], in1=xc[:, mt].bitcast(f32),
                        op0=OP.add, op1=OP.add)

        # ---- LN2 -> z2 and Phase B (MLP); pools coexist so LN2 chunks overlap
        # with the fc-weight DMA and early MLP matmuls ----
        with ExitStack() as post:
            zpool = post.enter_context(tc.tile_pool(name="zpool", bufs=1))
            l2 = post.enter_context(tc.tile_pool(name="l2", bufs=1))
            wB = post.enter_context(tc.tile_pool(name="wB", bufs=1))
            mb = post.enter_context(tc.tile_pool(name="mb", bufs=3))
            z2 = zpool.tile([P, CT, TOK_ALL], f32r)

            if True:
                ps_stats = post.enter_context(tc.tile_pool(name="ps_stats2", bufs=1, space="PSUM"))
                ps_bcast = post.enter_context(tc.tile_pool(name="ps_bcast2", bufs=1, space="PSUM"))
                for ch in range(NCHUNK):
                    tok0 = ch * T2
                    xsq = l2.tile([P, CT, T2], f32r, tag="xsq2")
                    for ct in range(CT):
                        nc.scalar.activation(out=xsq[:, ct],
                                             in_=x1[:, ct, tok0:tok0 + T2].bitcast(f32),
                                             func=AF.Square, bias=0.0, scale=1.0)
                    MU, R = _ln_stats(nc, rows, ps_stats, ps_bcast, onk, onm, eps11,
                                      [x1[:, ct, tok0:tok0 + T2] for ct in range(CT)],
                                      [xsq[:, ct] for ct in range(CT)])
                    R_sb = small.tile([P, T2], f32, tag="R_sb")
                    nc.scalar.activation(out=R_sb, in_=R, func=AF.Copy, bias=0.0, scale=1.0)
                    for ct in range(CT):
                        nc.vector.tensor_sub(z2[:, ct, tok0:tok0 + T2],
                                             x1[:, ct, tok0:tok0 + T2].bitcast(f32), MU)
                        nc.gpsimd.tensor_mul(z2[:, ct, tok0:tok0 + T2],
                                             z2[:, ct, tok0:tok0 + T2].bitcast(f32), R_sb)

            ps_m = post.enter_context(tc.tile_pool(name="ps_m", bufs=4, space="PSUM"))
            ps_o = ps_m

            # MLP: two 1536-wide chunks of fc1/fc2; m1 built in two halves of 6
            for j in range(NJ):
                # per-tile DMA pieces: the first matmuls of this j-chunk start
                # as soon as their own slice lands, not after the full 9.4MB
                fc1Wj = wB.tile([P, CT, JW], f32r, tag="fc1Wj")
                for kt in range(CT):
                    nc.sync.dma_start(
                        out=fc1Wj[:, kt],
                        in_=fc1W_d[:, j * JW:(j + 1) * JW]
                            .rearrange("(kt p) n -> p kt n", p=P)[:, kt].bitcast(f32r))
                fc2Wj = wB.tile([P, PT, DIM], f32r, tag="fc2Wj")
                for pt in range(PT):
                    nc.sync.dma_start(
                        out=fc2Wj[:, pt],
                        in_=fc2W_d[j * JW:(j + 1) * JW, :]
                            .rearrange("(pt p) n -> p pt n", p=P)[:, pt].bitcast(f32r))

                for ch in range(NCHUNK):
                    tok0 = ch * T2
                    halves = []
                    for h in range(2):
                        m1h = mb.tile([P, PT // 2, T2], f32r, tag="m1")
                        halves.append(m1h)
                        for pi in range(PT // 2):
                            pt = h * (PT // 2) + pi
                            pm = ps_m.tile([P, T2], f32, tag="pm")
                            for kt in range(CT):
                                nc.tensor.matmul(pm, fc1Wj[:, kt, pt * P:(pt + 1) * P],
                                                 z2[:, kt, tok0:tok0 + T2],
                                                 start=(kt == 0), stop=(kt == CT - 1))
                            nc.scalar.activation(out=m1h[:, pi], in_=pm, func=AF.Gelu,
                                                 bias=bfc1_sb[:, j * PT + pt:j * PT + pt + 1],
                                                 scale=1.0)
                    for mt in range(CT):
                        po = ps_o.tile([P, T2], f32, tag="pm")
                        for pt in range(PT):
                            nc.tensor.matmul(po, fc2Wj[:, pt, mt * P:(mt + 1) * P],
                                             halves[pt // (PT // 2)][:, pt % (PT // 2)],
                                             start=(pt == 0), stop=(pt == PT - 1))
                        nc.vector.tensor_add(x1[:, mt, tok0:tok0 + T2],
                                             x1[:, mt, tok0:tok0 + T2].bitcast(f32), po)
                        if j == NJ - 1:
                            # final value for this tile: stream it out now
                            nc.sync.dma_start(
                                out=yT_d.rearrange("(ct p) t -> p ct t", p=P)[:, mt, tok0:tok0 + T2],
                                in_=x1[:, mt, tok0:tok0 + T2].bitcast(f32))


def _prep_host(inputs):
    """Host-side preprocessing shared by all cores: fold LN affine params into
    weights, precompute bias vectors and the conv bias image."""
    g1 = inputs["ln1_g"].astype(np.float64)
    b1 = inputs["ln1_b"].astype(np.float64)
    g2 = inputs["ln2_g"].astype(np.float64)
    b2 = inputs["ln2_b"].astype(np.float64)
    kvW = inputs["kvW"].astype(np.float64)
    qW1 = inputs["qW1"].astype(np.float64)
    posW = inputs["posW"].astype(np.float64)      # (768,1,3,3)
    posB = inputs["posB"].astype(np.float64)
    fc1W = inputs["fc1W"].astype(np.float64)

    kvW_f = (g1[:, None] * kvW).astype(np.float32)
    qW1_f = (g1[:, None] * qW1).astype(np.float32)
    posW_f = (g1[:, None] * posW.reshape(DIM, 9)).astype(np.float32)
    fc1W_f = (g2[:, None] * fc1W).astype(np.float32)

    bias_kv = (b1 @ kvW).astype(np.float32)                  # (1536,)
    bias_u = (b1 @ qW1).astype(np.float32)                   # (64,)
    bias_fc1 = (b2 @ fc1W + inputs["fc1b"].astype(np.float64)).astype(np.float32)
    bias_prj = (inputs["projB"].astype(np.float64)
                + inputs["fc2b"].astype(np.float64)).astype(np.float32)

    # depthwise conv of the constant image b1 (per channel) + posB
    S = np.zeros((DIM, HW, HW), np.float64)
    for ky in range(3):
        for kx in range(3):
            y0, y1 = max(0, 1 - ky), min(HW, HW + 1 - ky)
            x0, x1_ = max(0, 1 - kx), min(HW, HW + 1 - kx)
            S[:, y0:y1, x0:x1_] += posW[:, 0, ky, kx][:, None, None]
    sepe_bias = (b1[:, None, None] * S + posB[:, None, None]).astype(np.float32)

    return {
        "kvW": np.ascontiguousarray(kvW_f),
        "qW1": np.ascontiguousarray(qW1_f),
        "qW2": np.ascontiguousarray(inputs["qW2"].astype(np.float32)),
        "projW": np.ascontiguousarray(inputs["projW"].astype(np.float32)),
        "fc1W": np.ascontiguousarray(fc1W_f),
        "fc2W": np.ascontiguousarray(inputs["fc2W"].astype(np.float32)),
        "posW": np.ascontiguousarray(posW_f),
        "sepeb": np.ascontiguousarray(sepe_bias.reshape(DIM, N_TOK)),
        "bkv": np.ascontiguousarray(bias_kv.reshape(MT_KV, P)),
        "bu": np.ascontiguousarray(bias_u.reshape(HID, 1)),
        "bprj": np.ascontiguousarray(bias_prj.reshape(CT, P)),
        "bfc1": np.ascontiguousarray(bias_fc1.reshape(MLP // P, P)),
        "ones": np.ones((P, 2), np.float32),
    }


def kernel(**inputs):
    if "nc" not in _CACHE:
        _CACHE["nc"] = build_module()
    nc = _CACHE["nc"]

    inputs = {k: np.asarray(v) for k, v in inputs.items()}
    shared = _prep_host(inputs)
    x = np.asarray(inputs["x"], dtype=np.float32)     # (64, 196, 768)

    in_maps = []
    for c in range(N_CORES):
        xc = x[c * B:(c + 1) * B].reshape(TOK_ALL, DIM)
        m = dict(shared)
        m["xT"] = np.ascontiguousarray(xc.T)          # (768, 1568)
        in_maps.append(m)

    res = run_bass_kernel_spmd(nc, in_maps, core_ids=list(range(N_CORES)))
    outs = []
    for c in range(N_CORES):
        yT = res.results[c]["yT"]                     # (768, 1568)
        outs.append(yT.T.reshape(B, N_TOK, DIM))
    return np.concatenate(outs, axis=0).astype(np.float32)

